# revision 23
# baseline (speedup 1.0000x reference)
"""Trainium2 Bass kernel for nn_BatchProgramCC (tree-CNN + BiGRU program-pair
classifier). Self-contained: hardcodes shapes/sharding; builds+runs an 8-core
SPMD Bass program via run_bass_kernel_spmd.

Sharding: data-parallel over B (8 programs/core); embedding table (bf16) and
all weights replicated.

Streaming architecture: the indirect-DMA gather is descriptor-rate-bound
(~1.4us per 128-row call, 256 calls/core) and owns the GpSimd queue for the
whole kernel, so everything else streams underneath it. Tokens are gathered
in 8 chunks per side; chunk j covers fwd stmts [8j,8j+8) and bwd stmts
[120-8j,128-8j), so after chunk j both GRU directions can advance steps
[8j,8j+8). Per chunk: gather (GpSimd) -> staging+transpose (Sync) ->
tree-sum + W_c + k-max (Vector/Tensor) -> enc scatter + n-gate GI (Scalar
copies) -> 8 GRU steps. The two sides run the GRU in lockstep with every op
batched across sides (one sigmoid [100,64], one tanh [100,32] per step), and
the z*h term is computed off the critical path. Final max-pool + fc +
softmax on device.
"""
import os
import numpy as np
import ml_dtypes

# ---- problem constants (hardcoded per contract) ----
B, S, K = 64, 128, 16
MAX_DEPTH = 5
V, E, H = 50000, 128, 100
NCORES = 8
BL = B // NCORES            # programs per core = 8
NT = BL * S * K             # tokens per core per side = 16384
NTREE = BL * S              # trees per core = 1024
NCHUNK = 8                  # stream chunks per side
CST = S // NCHUNK // 2      # fwd stmts per chunk = 8 (and 8 bwd)
CTOK = 2 * CST * BL * K     # tokens per chunk per side = 2048
NCALL = CTOK // 128         # gather calls per chunk per side = 16

# fixed binary-tree topology (matches reference._tree_structure)
_LOCAL_PARENT = np.array([0] + [(i - 1) // 2 for i in range(1, K)], dtype=np.int64)
_LOCAL_LEVEL = np.floor(np.log2(np.arange(K) + 1)).astype(np.int64)
# child-sum edge schedule, bottom-up: (parent, child) pairs in dependency order
_EDGES = [(7, 15),
          (3, 7), (3, 8), (4, 9), (4, 10), (5, 11), (5, 12), (6, 13), (6, 14),
          (1, 3), (1, 4), (2, 5), (2, 6),
          (0, 1), (0, 2)]


def _np_reference(tokens1, tokens2, parent, level, emb, W_c, b_c,
                  gru_wih_f, gru_whh_f, gru_bih_f, gru_bhh_f,
                  gru_wih_b, gru_whh_b, gru_bih_b, gru_bhh_b, fc_w, fc_b):
    """numpy fallback (used only if the inputs are not the fixed topology /
    zero-bias case this kernel specializes for)."""
    def sigmoid(x):
        return 1.0 / (1.0 + np.exp(-x))

    def gru_dir(x, w_ih, w_hh, b_ih, b_hh):
        b, s, e = x.shape
        h = np.zeros((b, w_hh.shape[1]), np.float32)
        ys = np.empty((b, s, w_hh.shape[1]), np.float32)
        for t in range(s):
            gi = x[:, t] @ w_ih.T + b_ih
            gh = h @ w_hh.T + b_hh
            ir, iz, inn = np.split(gi, 3, axis=1)
            hr, hz, hn = np.split(gh, 3, axis=1)
            r = sigmoid(ir + hr)
            z = sigmoid(iz + hz)
            n = np.tanh(inn + r * hn)
            h = (1.0 - z) * n + z * h
            ys[:, t] = h
        return ys

    def encode(tokens):
        h = emb[tokens] @ W_c.T + b_c
        for d in range(MAX_DEPTH - 1, 0, -1):
            contrib = np.where((level == d)[:, None], h, 0.0)
            np.add.at(h, parent, contrib)
        enc = np.maximum(h.reshape(B, S, K, E).max(axis=2), 0.0)
        fwd = gru_dir(enc, gru_wih_f, gru_whh_f, gru_bih_f, gru_bhh_f)
        bwd = gru_dir(enc[:, ::-1], gru_wih_b, gru_whh_b, gru_bih_b, gru_bhh_b)[:, ::-1]
        return np.concatenate([fwd, bwd], axis=-1).max(axis=1)

    lvec = encode(tokens1)
    rvec = encode(tokens2)
    y = np.concatenate([lvec, rvec], axis=1) @ fc_w.T + fc_b
    y = y - y.max(axis=1, keepdims=True)
    ey = np.exp(y)
    return (ey / ey.sum(axis=1, keepdims=True)).astype(np.float32)


def _build_program(fc_db):
    """Build the 8-core SPMD streaming Bass program."""
    import concourse.bacc as bacc
    import concourse.bass as bass
    import concourse.mybir as mybir
    import concourse.tile as tile

    f32 = mybir.dt.float32
    bf16 = mybir.dt.bfloat16
    i32 = mybir.dt.int32
    AL = mybir.AluOpType
    ACT = mybir.ActivationFunctionType

    nc = bacc.Bacc()

    # ---- DRAM tensors ----
    emb_d = nc.dram_tensor("embbf", [V, E], bf16, kind="ExternalInput")
    # idx col = (chunk*2 + side)*NCALL + call; call c gathers k=c for the
    # chunk's 16 stmts x 8 progs; partition p = s16*8 + prog
    idx_d = nc.dram_tensor("idx", [128, 2 * NCHUNK * NCALL], i32,
                           kind="ExternalInput")
    wct_d = nc.dram_tensor("wct", [128, 128], bf16, kind="ExternalInput")
    wih_d = nc.dram_tensor("wih", [128, 6 * 128], bf16, kind="ExternalInput")
    whh_d = nc.dram_tensor("whh", [128, 6 * 128], bf16, kind="ExternalInput")
    fcw_d = nc.dram_tensor("fcw", [128, 4 * 2], bf16, kind="ExternalInput")
    out_d = nc.dram_tensor("out", [BL, 2], f32, kind="ExternalOutput")

    with tile.TileContext(nc) as tc:
        with tc.tile_pool(name="const", bufs=1) as cpool, \
             tc.tile_pool(name="gf", bufs=12) as gf_pool, \
             tc.tile_pool(name="xc", bufs=3) as xc_pool, \
             tc.tile_pool(name="mx", bufs=2) as mx_pool, \
             tc.tile_pool(name="encp", bufs=1) as enc_pool, \
             tc.tile_pool(name="stepp", bufs=8) as step_pool, \
             tc.tile_pool(name="psum_big", bufs=2, space="PSUM") as ps_big, \
             tc.tile_pool(name="psum_gi", bufs=2, space="PSUM") as ps_gi, \
             tc.tile_pool(name="psum_gru", bufs=3, space="PSUM") as ps_gru, \
             tc.tile_pool(name="dram", bufs=4, space="DRAM") as dram_pool:

            # ---- constants ----
            idx_t = cpool.tile([128, 2 * NCHUNK * NCALL], i32, name="idx_t")
            nc.sync.dma_start(idx_t[:], idx_d[:])
            wct = cpool.tile([128, 128], bf16, name="wct_t")
            nc.sync.dma_start(wct[:], wct_d[:])
            wih = cpool.tile([128, 6 * 128], bf16, name="wih_t")
            nc.sync.dma_start(wih[:], wih_d[:])
            whh = cpool.tile([128, 6 * 128], bf16, name="whh_t")
            nc.sync.dma_start(whh[:], whh_d[:])
            fcw = cpool.tile([128, 8], bf16, name="fcw_t")
            nc.sync.dma_start(fcw[:], fcw_d[:])

            # ---- persistent state tiles ----
            # enc combined: col = stmt*16 + side*8 + prog, bf16
            encc = enc_pool.tile([128, S * 16], bf16, name="encc")
            # n-gate input: col = t*32 + [nf s0|s1 (16) | nb s0|s1 (16)]
            GI = enc_pool.tile([128, S * 32], bf16, name="GI")
            GIv = GI.rearrange("p (t c) -> p t c", t=S)
            # h history: slot t cols = [s0f(8) s1f(8) s0b(8) s1b(8)]
            Hb = enc_pool.tile([128, (S + 1) * 32], bf16, name="Hb")
            nc.vector.memset(Hb[:], 0)
            Hv = Hb.rearrange("p (t c) -> p t c", c=32)

            # gate blocks in wih/whh: 0=r_f 1=r_b 2=z_f(neg) 3=z_b(neg)
            #                         4=n_f 5=n_b
            # per-step PSUM [128, 96]: r 0:16 (f: s0|s1), r_b 16:32,
            #   wait -- layout: 0:16 r_f(s0,s1) 16:32 r_b 32:48 z'_f
            #   48:64 z'_b 64:80 n_f 80:96 n_b ; sigma reads 0:64
            mm_list = [(0, 0, 0), (1, 16, 16), (2, 32, 0), (3, 48, 16),
                       (4, 64, 0), (5, 80, 16)]

            def gru_step(t, qp_eng):
                tb = S - 1 - t
                hprev = Hv[:, t, :]
                ps = ps_gru.tile([128, 96], f32, tag="gru")
                for g, col, hc in mm_list:
                    tg = t if g in (0, 2) else (tb if g in (1, 3) else None)
                    nc.tensor.matmul(
                        ps[:, col:col + 16], whh[:, g * 128:(g + 1) * 128],
                        hprev[:, hc:hc + 16], start=True,
                        stop=(tg is None))
                    if tg is not None:
                        nc.tensor.matmul(
                            ps[:, col:col + 16],
                            wih[:, g * 128:(g + 1) * 128],
                            encc[:, tg * 16:(tg + 1) * 16],
                            start=False, stop=True)
                rz = step_pool.tile([128, 64], bf16, tag="rz")
                nc.scalar.activation(rz[0:100, :], ps[0:100, 0:64],
                                     ACT.Sigmoid)
                # q = z'*h ; p = h - q = z*h  (off the critical path)
                q_ = step_pool.tile([128, 32], bf16, tag="q_")
                qp_eng.tensor_tensor(out=q_[0:100, :],
                                     in0=rz[0:100, 32:64],
                                     in1=hprev[0:100, :], op=AL.mult)
                p_ = step_pool.tile([128, 32], bf16, tag="p_")
                qp_eng.tensor_tensor(out=p_[0:100, :],
                                     in0=hprev[0:100, :],
                                     in1=q_[0:100, :], op=AL.subtract)
                m_ = step_pool.tile([128, 32], bf16, tag="m_")
                nc.vector.tensor_tensor(out=m_[0:100, :],
                                        in0=ps[0:100, 64:96],
                                        in1=rz[0:100, 0:32], op=AL.mult)
                av = step_pool.tile([128, 32], bf16, tag="av")
                nc.vector.tensor_tensor(out=av[0:100, :], in0=m_[0:100, :],
                                        in1=GIv[0:100, t, :], op=AL.add)
                nt_ = step_pool.tile([128, 32], bf16, tag="nt")
                nc.scalar.activation(nt_[0:100, :], av[0:100, :], ACT.Tanh)
                w_ = step_pool.tile([128, 32], bf16, tag="w_")
                nc.vector.tensor_tensor(out=w_[0:100, :],
                                        in0=nt_[0:100, :],
                                        in1=rz[0:100, 32:64], op=AL.mult)
                nc.vector.tensor_tensor(out=Hv[0:100, t + 1, :],
                                        in0=w_[0:100, :],
                                        in1=p_[0:100, :], op=AL.add)

            def emit_supply(j):
                """Gathers + staging + transpose + tree-sum + W_c + enc + GI
                for chunk j."""
                gfs = []
                for s in range(2):
                    gf = gf_pool.tile([128, NCALL, 128], bf16, tag="gf")
                    for c in range(NCALL):
                        col = (j * 2 + s) * NCALL + c
                        nc.gpsimd.indirect_dma_start(
                            out=gf[:, c, :], out_offset=None, in_=emb_d[:],
                            in_offset=bass.IndirectOffsetOnAxis(
                                ap=idx_t[:, col:col + 1], axis=0))
                    gfs.append(gf)
                Xcs = []
                for s in range(2):
                    stg = dram_pool.tile([CTOK, E], bf16, tag="stg")
                    nc.sync.dma_start(
                        stg[:].rearrange("(c p) e -> p c e", p=128),
                        gfs[s][:])
                    Xc = xc_pool.tile([128, CTOK], bf16, tag="xc")
                    nc.sync.dma_start_transpose(Xc[:], stg[:])
                    Xcs.append(Xc)
                for s in range(2):
                    Xc = Xcs[s]
                    # tree child-sum: k-slices of 128 cols
                    Xk = Xc.rearrange("p (k n) -> p k n", k=K)
                    for (pn, cn) in _EDGES:
                        nc.vector.tensor_tensor(
                            out=Xk[:, pn, :], in0=Xk[:, pn, :],
                            in1=Xk[:, cn, :], op=AL.add)
                    # W_c matmuls: 4 x 512 cols -> mxp f32
                    mxp = mx_pool.tile([128, CTOK], f32, tag="mx")
                    for q in range(CTOK // 512):
                        ps = ps_big.tile([128, 512], f32, tag="wc")
                        nc.tensor.matmul(ps[:], wct[:],
                                         Xc[:, q * 512:(q + 1) * 512],
                                         start=True, stop=True)
                        nc.vector.tensor_copy(mxp[:, q * 512:(q + 1) * 512],
                                              ps[:])
                    # max over k: halving reduction on [128, 2048]
                    w = CTOK // 2
                    while w >= 128:
                        nc.vector.tensor_tensor(
                            out=mxp[:, 0:w], in0=mxp[:, 0:w],
                            in1=mxp[:, w:2 * w], op=AL.max)
                        w //= 2
                    # relu + scatter into encc: src cols = s16*8 + prog,
                    # fwd u<8 -> stmt 8j+u ; bwd u>=8 -> stmt 120-8j+(u-8)
                    srcv = mxp[:, 0:128].rearrange("p (u b) -> p u b", u=16)
                    for half, base_st in ((0, CST * j), (1, (S - CST * (j + 1)))):
                        dst = encc.rearrange("p (t c) -> p t c", t=S)[
                            :, base_st:base_st + CST, s * BL:(s + 1) * BL]
                        nc.vector.tensor_scalar(
                            out=dst, in0=srcv[:, half * CST:(half + 1) * CST, :],
                            scalar1=0.0, scalar2=None, op0=AL.max)
                # n-gate GI: each stmt window feeds a fwd slot block
                # (nf, t=stmt) AND a bwd slot block (nb, t=127-stmt, reversed)
                for base_st in (CST * j, S - CST * (j + 1)):
                    for g in (4, 5):
                        psn = ps_gi.tile([128, CST * 16], f32, tag="gi")
                        nc.tensor.matmul(
                            psn[:], wih[:, g * 128:(g + 1) * 128],
                            encc[:, base_st * 16:(base_st + CST) * 16],
                            start=True, stop=True)
                        psv = psn.rearrange("p (t c) -> p t c", t=CST)
                        if g == 4:
                            dst = GIv[:, base_st:base_st + CST, 0:16]
                        else:
                            lo = S - 1 - base_st - (CST - 1)
                            dst = GIv[:, lo:lo + CST, 16:32][:, ::-1, :]
                        nc.scalar.copy(dst, psv[:])

            # software-pipelined emission: chunk j+1's supply ops precede
            # chunk j's GRU steps in every engine queue, so a stalled step
            # chain never blocks the gather stream
            emit_supply(0)
            for j in range(NCHUNK):
                if j + 1 < NCHUNK:
                    emit_supply(j + 1)
                for u in range(CST):
                    gru_step(CST * j + u, qp_eng=nc.vector)

            # ---- remaining 64 steps: all stmt data already on chip; the
            # gather queue is drained, so z*h moves to gpsimd ----
            for t in range(S // 2, S):
                gru_step(t, qp_eng=nc.gpsimd)

            # ---- max-pool over time: tree reduction on the h history ----
            n = S
            base = 1
            while n > 1:
                half = n // 2
                nc.vector.tensor_tensor(
                    out=Hv[0:100, base:base + half, :],
                    in0=Hv[0:100, base:base + half, :],
                    in1=Hv[0:100, base + half:base + 2 * half, :], op=AL.max)
                n = half
            pooled = Hv[:, base, :]   # cols [s0f s1f s0b s1b]

            # ---- fc + softmax ----
            # fc chunks over the 4H concat: 0=fwd_L 1=bwd_L 2=fwd_R 3=bwd_R
            psf = ps_gi.tile([128, 8], f32, tag="fc", bufs=1)
            chunks = [0, 16, 8, 24]   # pooled col offsets per fc chunk
            for ci, col in enumerate(chunks):
                nc.tensor.matmul(
                    psf[0:2, :], fcw[:, ci * 2:(ci + 1) * 2],
                    pooled[:, col:col + 8],
                    start=(ci == 0), stop=(ci == 3))
            t32 = step_pool.tile([128, 32], f32, tag="t32")
            nc.vector.memset(t32[0:32, :], 0)
            nc.vector.tensor_copy(t32[0:2, 0:8], psf[0:2, :])
            t32b = step_pool.tile([128, 32], f32, tag="t32b")
            nc.vector.transpose(t32b[0:32, :], t32[0:32, :])
            dcol = step_pool.tile([128, 2], f32, tag="dcol")
            nc.vector.tensor_tensor(out=dcol[0:8, 0:1], in0=t32b[0:8, 0:1],
                                    in1=t32b[0:8, 1:2], op=AL.subtract)
            outt = step_pool.tile([128, 2], f32, tag="outt")
            nc.scalar.activation(outt[0:8, 0:1], dcol[0:8, 0:1], ACT.Sigmoid,
                                 bias=float(fc_db))
            nc.vector.tensor_scalar(
                out=outt[0:8, 1:2], in0=outt[0:8, 0:1], scalar1=-1.0, scalar2=1.0,
                op0=AL.mult, op1=AL.add)
            nc.sync.dma_start(out_d[:], outt[0:8, 0:2])

    nc.compile()
    return nc


_CACHED = {}


def _chunk_stmts(j):
    """stmt list (len 16) for chunk j: 8 fwd-window + 8 bwd-window stmts."""
    return list(range(CST * j, CST * (j + 1))) + \
        list(range(S - CST * (j + 1), S - CST * j))


def kernel(**inputs):
    inputs = {k: np.asarray(v) for k, v in inputs.items()}
    tokens1 = inputs["tokens1"].astype(np.int64)
    tokens2 = inputs["tokens2"].astype(np.int64)
    parent = inputs["parent"].astype(np.int64)
    level = inputs["level"].astype(np.int64)
    emb = inputs["emb"].astype(np.float32)
    W_c = inputs["W_c"].astype(np.float32)
    b_c = inputs["b_c"].astype(np.float32)
    fc_w = inputs["fc_w"].astype(np.float32)
    fc_b = inputs["fc_b"].astype(np.float32)
    gw = {k: inputs[k].astype(np.float32) for k in (
        "gru_wih_f", "gru_whh_f", "gru_bih_f", "gru_bhh_f",
        "gru_wih_b", "gru_whh_b", "gru_bih_b", "gru_bhh_b")}

    # verify the fixed topology / zero-bias case this kernel specializes for
    base = np.arange(B * S, dtype=np.int64)[:, None] * K
    exp_parent = (base + _LOCAL_PARENT[None, :]).reshape(-1)
    exp_level = np.tile(_LOCAL_LEVEL, B * S)
    zero_bias = not (b_c.any() or any(
        gw[k].any() for k in ("gru_bih_f", "gru_bhh_f", "gru_bih_b",
                              "gru_bhh_b")))
    if not (np.array_equal(parent, exp_parent)
            and np.array_equal(level, exp_level) and zero_bias):
        return _np_reference(tokens1, tokens2, parent, level, emb, W_c, b_c,
                             gw["gru_wih_f"], gw["gru_whh_f"], gw["gru_bih_f"],
                             gw["gru_bhh_f"], gw["gru_wih_b"], gw["gru_whh_b"],
                             gw["gru_bih_b"], gw["gru_bhh_b"], fc_w, fc_b)

    # ---- host-side weight packing (layout prep only) ----
    bf = ml_dtypes.bfloat16
    embbf = emb.astype(bf)
    wct = np.ascontiguousarray(W_c.T).astype(bf)                   # [128,128] lhsT

    # gate order: 0=r_f 1=r_b 2=z_f(neg) 3=z_b(neg) 4=n_f 5=n_b
    def pack_w(w, negate):  # w [100, D] -> [D, 128] lhsT padded
        out = np.zeros((w.shape[1], 128), np.float32)
        out[:, :100] = w.T * (-1.0 if negate else 1.0)
        return out
    gates = [("f", 0, False), ("b", 0, False), ("f", 1, True),
             ("b", 1, True), ("f", 2, False), ("b", 2, False)]
    wih = np.concatenate(
        [pack_w(gw[f"gru_wih_{d}"][gi * H:(gi + 1) * H], neg)
         for d, gi, neg in gates], axis=1).astype(bf)               # [128, 6*128]
    whh_full = np.concatenate(
        [pack_w(gw[f"gru_whh_{d}"][gi * H:(gi + 1) * H], neg)
         for d, gi, neg in gates], axis=1)                          # [100, 6*128]
    whh = np.zeros((128, 6 * 128), np.float32)
    whh[:H] = whh_full
    whh = whh.astype(bf)
    # fc chunks: 0=fwd_L 1=bwd_L 2=fwd_R 3=bwd_R (order of reference concat
    # [lvec fwd|bwd, rvec fwd|bwd])
    fcw = np.zeros((128, 8), np.float32)
    for ci in range(4):
        fcw[:H, ci * 2:(ci + 1) * 2] = fc_w[:, ci * H:(ci + 1) * H].T
    fcw = fcw.astype(bf)
    fc_db = float(fc_b[0] - fc_b[1])

    # ---- per-core token index array ----
    # idx col = (chunk*2 + side)*NCALL + c(k); partition p = s16*8 + prog
    def idx_for(tokens, core):
        t3 = tokens.reshape(B, S, K)[core * BL:(core + 1) * BL]    # [8,128,16]
        cols = np.empty((128, NCHUNK * NCALL), np.int32)
        for j in range(NCHUNK):
            stmts = _chunk_stmts(j)
            blk = t3[:, stmts, :]                                   # [8,16,16]
            for c in range(NCALL):
                # partition p = s16*8 + prog
                colv = blk[:, :, c].T.reshape(-1)                   # s16-major
                cols[:, j * NCALL + c] = colv
        return cols

    from concourse.bass_utils import run_bass_kernel_spmd

    key = ("prog", fc_db)
    if key not in _CACHED:
        _CACHED[key] = _build_program(fc_db)
    nc = _CACHED[key]

    in_maps = []
    for c in range(NCORES):
        i1 = idx_for(tokens1, c)
        i2 = idx_for(tokens2, c)
        idx = np.empty((128, 2 * NCHUNK * NCALL), np.int32)
        for j in range(NCHUNK):
            idx[:, (j * 2) * NCALL:(j * 2 + 1) * NCALL] = \
                i1[:, j * NCALL:(j + 1) * NCALL]
            idx[:, (j * 2 + 1) * NCALL:(j * 2 + 2) * NCALL] = \
                i2[:, j * NCALL:(j + 1) * NCALL]
        in_maps.append({"embbf": embbf, "idx": idx, "wct": wct, "wih": wih,
                        "whh": whh, "fcw": fcw})

    if os.environ.get("BPCC_SIM"):
        from concourse.bass_interp import CoreSim
        sim = CoreSim(nc)
        for k, v in in_maps[0].items():
            sim.tensor(k)[:] = v
        sim.simulate()
        o0 = np.asarray(sim.tensor("out")).copy()
        return np.vstack([o0] * NCORES).astype(np.float32)

    trace = bool(os.environ.get("BPCC_TRACE"))
    if trace:
        try:
            import axon_prof_shim  # noqa: F401
        except ImportError:
            trace = False
    res = run_bass_kernel_spmd(nc, in_maps, core_ids=list(range(NCORES)),
                               trace=trace)
    if trace and res.exec_time_ns:
        print(f"HW exec time: {res.exec_time_ns} ns")
    out = np.vstack([res.results[c]["out"] for c in range(NCORES)])
    return out.astype(np.float32)
